# revision 24
# baseline (speedup 1.0000x reference)
"""MixtureRouter Trainium2 kernel (final).

Per-core (data-parallel over batch, 8 cores): LayerNorm + Linear(2048->512)
+ GELU + sum-over-sequence, emitting the per-core GELU accumulator
[128, 512] (token-slot x feature). Host sums the 128 token-slot rows,
applies the tiny tail (H @ w2 + S*b2 -> router head) and computes
aux_loss / next_idx in fp64 (trivial 512x512 math, exact rewrite).

Key design points:
  - x is rounded to bf16 on the host (round-to-nearest, symmetric error
    ~2^-9; measured end-to-end logits error ~3e-4 vs the 2e-2 gate).
    bf16 halves HBM traffic (8 MiB/core), makes the PE transposes
    single-pass, and runs the 256 matmuls at the same 1 cyc/row as f32r
    with less power draw (dense fp32 PE work on all 8 cores trips a chip
    power throttle 2.4 -> ~2.0 GHz; measured).
  - LayerNorm folds into the matmul output: with pm = x @ w1g,
      pre = r*(pm - mu (x) u + sqrt(var+eps) (x) vb1)
    i.e. ONE K=2 rank-2 matmul into the same PSUM bank (stationary rows
    [-mu_t, sqrt(v)_t], moving rows [u, vb1]), and the r-scale rides the
    ACT Gelu as a per-partition `scale` operand. No elementwise
    correction passes at all.
  - Stats (mu, var) via DVE bn_stats directly on the bf16 tiles.
  - sum-over-tokens commutes with the second linear:
      sum_t(gelu_h @ w2 + b2) = (sum_t gelu_h) @ w2 + S*b2  (host).
  - rsqrt via DVE bit-trick + 2 Newton steps (avoids ACT table thrash).
  - PE software-pipelined one tile ahead (transposes of tile i+1 issued
    before matmuls of tile i) so the PSUM->SBUF copies (ACT) never stall
    it. w1g is uploaded pre-rearranged so its DMA uses 16KB descriptors.
"""

import sys
import types

import ml_dtypes
import numpy as np

import concourse.bass as bass
import concourse.mybir as mybir
import concourse.tile as tile
from concourse import bacc
from concourse.bass_utils import run_bass_kernel_spmd
from concourse.masks import make_identity

# run_bass_kernel_spmd imports antenv.axon_hooks when BASS_TRACE is set; that
# module is absent on this image. Provide it so tracing degrades gracefully.
if "antenv.axon_hooks" not in sys.modules:
    try:
        import antenv.axon_hooks  # noqa: F401
    except ImportError:
        _hm = types.ModuleType("antenv.axon_hooks")
        _hm._hook = None
        _hm.set_axon_ntff_profile_hook = lambda h: setattr(_hm, "_hook", h)
        _hm.get_axon_ntff_profile_hook = lambda: _hm._hook
        sys.modules["antenv.axon_hooks"] = _hm
        try:
            from trn_agent_boot.trn_boot import _ntff_profile_via_ctypes

            _hm._hook = _ntff_profile_via_ctypes("/opt/axon/libaxon_pjrt.so")
        except Exception:
            pass

F32 = mybir.dt.float32
F32R = mybir.dt.float32r
BF16 = mybir.dt.bfloat16
I32 = mybir.dt.int32

B, S, D, R, E = 8, 2048, 2048, 512, 8
N_CORES = 8
P = 128
NT = S // P          # 16 token tiles per core
NK = D // P          # 16 contraction chunks
GRP = 4              # stat-processing group (tiles)
LN_EPS = 1e-5

_cache = {}


def _build():
    nc = bacc.Bacc("TRN2", target_bir_lowering=False, debug=False, num_devices=N_CORES)
    xb = nc.dram_tensor("xb", [S, D], BF16, kind="ExternalInput")
    w1g = nc.dram_tensor("w1g", [P, NK * R], BF16, kind="ExternalInput")
    uvb = nc.dram_tensor("uvb", [2, R], BF16, kind="ExternalInput")
    gout = nc.dram_tensor("gout", [P, R], F32, kind="ExternalOutput")

    with tile.TileContext(nc) as tc:
        with (
            tc.tile_pool(name="const", bufs=1) as const,
            tc.tile_pool(name="xin", bufs=3) as xin,
            tc.tile_pool(name="xtp", bufs=2) as xtp,
            tc.tile_pool(name="stats", bufs=2) as stats,
            tc.tile_pool(name="rows", bufs=2) as rows,
            tc.tile_pool(name="work", bufs=3) as work,
            tc.tile_pool(name="pst", bufs=2, space="PSUM") as pst,
            tc.tile_pool(name="psm", bufs=5, space="PSUM") as psm,
            tc.tile_pool(name="pssml", bufs=1, space="PSUM") as pssml,
        ):
            # ---- startup-critical DMA ordering: x tile 0 halves first.
            xs_tiles = [None] * NT
            xs0 = xin.tile([P, NK, P], BF16, tag="xs")
            for q4 in range(4):
                nc.sync.dma_start(
                    xs0[:, q4 * 4 : (q4 + 1) * 4],
                    xb[0:P, q4 * D // 4 : (q4 + 1) * D // 4].rearrange(
                        "p (k q) -> p k q", q=P
                    ),
                )
            xs_tiles[0] = xs0

            ident = const.tile([P, P], BF16)
            make_identity(nc, ident)

            xs1 = xin.tile([P, NK, P], BF16, tag="xs")
            for h2 in range(2):
                nc.sync.dma_start(
                    xs1[:, h2 * 8 : (h2 + 1) * 8],
                    xb[P : 2 * P, h2 * D // 2 : (h2 + 1) * D // 2].rearrange(
                        "p (k q) -> p k q", q=P
                    ),
                )
            xs_tiles[1] = xs1

            # w1g pre-rearranged on host: w1g[p, k*R+n] = w1g_logical[k*128+p, n]
            # (triggered after the first two x tiles so their descriptors win
            # the early engine-bandwidth race; w1gA is only needed ~16us in)
            w1gA = const.tile([P, 4, R], BF16)
            nc.scalar.dma_start(
                w1gA, w1g[:, : 4 * R].rearrange("p (k n) -> p k n", n=R)
            )
            w1gB = const.tile([P, NK - 4, R], BF16)
            nc.scalar.dma_start(
                w1gB, w1g[:, 4 * R :].rearrange("p (k n) -> p k n", n=R)
            )

            def w1g_chunk(k):
                return w1gA[:, k] if k < 4 else w1gB[:, k - 4]

            uvb_s = const.tile([2, R], BF16)
            nc.scalar.dma_start(uvb_s, uvb[:, :])

            gacc = const.tile([P, R], F32)
            nc.vector.memset(gacc, 0.0)

            # ---- per-tile emission, software-pipelined one tile ahead on PE
            mu_g = vr_g = None
            pm_tiles = [None] * NT
            xT_tiles = [None] * NT
            group_ready = [None] * (NT // GRP)   # (rg, row2) per group
            pending = []                         # tiles awaiting K2+postproc

            def emit_front(ti):
                """DMA (ti+2), stats(ti), transposes(ti) + copies(ti)."""
                if ti + 2 < NT:
                    xs_n = xin.tile([P, NK, P], BF16, tag="xs")
                    nc.sync.dma_start(
                        xs_n,
                        xb[(ti + 2) * P : (ti + 3) * P, :].rearrange(
                            "p (k q) -> p k q", q=P
                        ),
                    )
                    xs_tiles[ti + 2] = xs_n
                xs = xs_tiles[ti]

                st = stats.tile([P, 4, 6], F32, tag="bn")
                xs4 = xs.rearrange("p (a k) q -> p a (k q)", a=4)
                for c in range(4):
                    nc.vector.bn_stats(st[:, c], xs4[:, c])
                mv = stats.tile([P, 2], F32, tag="mv")
                nc.vector.bn_aggr(mv, st)
                gi = ti % GRP
                nc.vector.tensor_copy(mu_g[:, gi : gi + 1], mv[:, 0:1])
                nc.vector.tensor_copy(vr_g[:, gi : gi + 1], mv[:, 1:2])

                xT = xtp.tile([P, NK, P], BF16, tag="xT")
                for kg in range(4):
                    pt = pst.tile([P, 4, P], BF16, tag="tp")
                    for j in range(4):
                        k = kg * 4 + j
                        nc.tensor.transpose(pt[:, j], xs[:, k], ident)
                    nc.scalar.copy(xT[:, kg * 4 : (kg + 1) * 4], pt)
                xT_tiles[ti] = xT

            def emit_matmuls(ti):
                pm = psm.tile([P, R], F32, tag="mm")
                xT = xT_tiles[ti]
                for k in range(NK):
                    nc.tensor.matmul(
                        pm, xT[:, k], w1g_chunk(k),
                        start=(k == 0), stop=False, skip_group_check=True,
                    )
                pm_tiles[ti] = pm

            def emit_group_math():
                """rsqrt + stat-row transposes for the current group."""
                v = stats.tile([P, GRP], F32, tag="v")
                nc.vector.tensor_scalar_add(v, vr_g, LN_EPS)
                r = stats.tile([P, GRP], F32, tag="r")
                nc.vector.tensor_scalar(
                    r.bitcast(I32), v.bitcast(I32), 1, None,
                    op0=mybir.AluOpType.arith_shift_right,
                )
                nc.vector.tensor_scalar(
                    r.bitcast(I32), r.bitcast(I32), 0x5F3759DF, -1,
                    op0=mybir.AluOpType.subtract, op1=mybir.AluOpType.mult,
                )
                t = stats.tile([P, GRP], F32, tag="t")
                for _ in range(2):
                    nc.vector.tensor_mul(t, r, r)
                    nc.vector.tensor_mul(t, t, v)
                    nc.vector.tensor_scalar(
                        t, t, -0.5, 1.5,
                        op0=mybir.AluOpType.mult, op1=mybir.AluOpType.add,
                    )
                    nc.vector.tensor_mul(r, r, t)
                rg = stats.tile([P, GRP], F32, tag="rg")
                nc.vector.tensor_copy(rg, r)
                # stationary rows for the K=2 correction: [-mu_j, sqrt(v)_j]
                # (bf16 rows; the correction is small relative to pre, so
                # bf16 rounding of it is ~5e-5 absolute -- negligible).
                nmsq = stats.tile([P, 2, GRP], BF16, tag="nmsq")
                nc.vector.tensor_scalar_mul(nmsq[:, 0], mu_g, -1.0)
                nc.vector.tensor_mul(nmsq[:, 1], v, rg)
                strow = pssml.tile([2, GRP, P], BF16, tag="strow")
                for j in range(GRP):
                    nc.tensor.transpose(strow[:, j], nmsq[:, :, j], ident)
                row2 = rows.tile([2, GRP, P], BF16, tag="row2")
                nc.vector.tensor_copy(row2, strow)
                return rg, row2

            def emit_postproc(ti):
                """K2 rank-2 correction (PE), Gelu*r (ACT), accumulate (DVE)."""
                gi = ti % GRP
                rg, row2 = group_ready[ti // GRP]
                nc.tensor.matmul(
                    pm_tiles[ti], row2[:, gi], uvb_s,
                    start=False, stop=True, skip_group_check=True,
                )
                gt = work.tile([P, R], F32, tag="g")
                nc.scalar.activation(
                    gt, pm_tiles[ti], mybir.ActivationFunctionType.Gelu,
                    scale=rg[:, gi : gi + 1],
                )
                pm_tiles[ti] = None
                nc.vector.tensor_add(gacc, gacc, gt)

            def flush_postproc(upto=None):
                # Hold K2s back ~2 tiles behind the newest W-block so the
                # DVE group-math -> row2 chain never stalls the PE.
                while pending and group_ready[pending[0] // GRP] is not None and (
                    upto is None or pending[0] <= upto
                ):
                    emit_postproc(pending.pop(0))

            for ti in range(NT):
                if ti % GRP == 0:
                    mu_g = stats.tile([P, GRP], F32, tag="mu")
                    vr_g = stats.tile([P, GRP], F32, tag="vr")
                emit_front(ti)
                if ti > 0:
                    emit_matmuls(ti - 1)
                    pending.append(ti - 1)
                if ti % GRP == GRP - 1:
                    group_ready[ti // GRP] = emit_group_math()
                flush_postproc(upto=ti - 3)
            emit_matmuls(NT - 1)
            pending.append(NT - 1)
            flush_postproc()
            assert not pending

            nc.scalar.dma_start(gout[:, :], gacc)
    nc.finalize()
    return nc


def kernel(hidden_states, ln_gamma, ln_beta, w1, b1, w2, b2, wr, br):
    hs = np.asarray(hidden_states, dtype=np.float32)
    xb = np.asarray(hs, dtype=ml_dtypes.bfloat16)            # round-to-nearest
    g64 = np.asarray(ln_gamma, dtype=np.float64)
    be64 = np.asarray(ln_beta, dtype=np.float64)
    w1_64 = np.asarray(w1, dtype=np.float64)
    w1g = (g64[:, None] * w1_64).astype(np.float32)
    w1gb = np.asarray(w1g, dtype=ml_dtypes.bfloat16)
    # u must match the device matmul's weights (bf16) exactly
    u = w1gb.astype(np.float64).sum(0).astype(np.float32)
    # device layout: w1gh[p, k*R+n] = w1g[k*128+p, n]  (16KB contiguous rows)
    w1gh = np.ascontiguousarray(
        w1gb.reshape(16, 128, 512).transpose(1, 0, 2).reshape(128, 16 * 512)
    )
    vb1 = (be64 @ w1_64 + np.asarray(b1, np.float64)).astype(np.float32)
    uvb = np.asarray(np.stack([u, vb1], axis=0), dtype=ml_dtypes.bfloat16)  # [2, R]

    if "nc" not in _cache:
        _cache["nc"] = _build()
    nc = _cache["nc"]

    in_maps = []
    for b in range(N_CORES):
        in_maps.append({
            "xb": np.ascontiguousarray(xb[b]),
            "w1g": w1gh, "uvb": uvb,
        })
    res = run_bass_kernel_spmd(nc, in_maps, core_ids=list(range(N_CORES)))
    gaccs = np.stack([res.results[b]["gout"] for b in range(N_CORES)], axis=0)
    global _last_res
    _last_res = res

    # host tail in fp64 (tiny): H -> w2 -> router -> aux/next_idx
    H = gaccs.astype(np.float64).sum(axis=1)                      # [B, R]
    bt = H @ np.asarray(w2, np.float64) + float(S) * np.asarray(b2, np.float64)
    logits = bt @ np.asarray(wr, np.float64) + np.asarray(br, np.float64)  # [B, E]
    global _last_logits
    _last_logits = logits.astype(np.float32)

    idx = logits.argmax(axis=-1)
    targets = np.zeros_like(logits)
    targets[np.arange(B), idx] = 1.0
    aux = (np.logaddexp(0.0, logits) - logits * targets).mean()
    counts = targets.sum(0)
    next_idx = int(np.argmax(counts))
    return np.float32(aux), np.int32(next_idx)


# revision 25
# speedup vs baseline: 1.0245x; 1.0245x over previous
"""MixtureRouter Trainium2 kernel (final).

Per-core (data-parallel over batch, 8 cores): LayerNorm + Linear(2048->512)
+ GELU + sum-over-sequence, emitting the per-core GELU accumulator
[128, 512] (token-slot x feature). Host sums the 128 token-slot rows,
applies the tiny tail (H @ w2 + S*b2 -> router head) and computes
aux_loss / next_idx in fp64 (trivial 512x512 math, exact rewrite).

Key design points:
  - x is rounded to bf16 on the host (round-to-nearest, symmetric error
    ~2^-9; measured end-to-end logits error ~3e-4 vs the 2e-2 gate).
    bf16 halves HBM traffic (8 MiB/core), makes the PE transposes
    single-pass, and runs the 256 matmuls at the same 1 cyc/row as f32r
    with less power draw (dense fp32 PE work on all 8 cores trips a chip
    power throttle 2.4 -> ~2.0 GHz; measured).
  - LayerNorm folds into the matmul output: with pm = x @ w1g,
      pre = r*(pm - mu (x) u + sqrt(var+eps) (x) vb1)
    i.e. ONE K=2 rank-2 matmul into the same PSUM bank (stationary rows
    [-mu_t, sqrt(v)_t], moving rows [u, vb1]), and the r-scale rides the
    ACT Gelu as a per-partition `scale` operand. No elementwise
    correction passes at all.
  - Stats (mu, var) via DVE bn_stats directly on the bf16 tiles.
  - sum-over-tokens commutes with the second linear:
      sum_t(gelu_h @ w2 + b2) = (sum_t gelu_h) @ w2 + S*b2  (host).
  - rsqrt via DVE bit-trick + 2 Newton steps (avoids ACT table thrash).
  - PE software-pipelined one tile ahead (transposes of tile i+1 issued
    before matmuls of tile i) so the PSUM->SBUF copies (ACT) never stall
    it. w1g is uploaded pre-rearranged so its DMA uses 16KB descriptors.
"""

import sys
import types

import ml_dtypes
import numpy as np

import concourse.bass as bass
import concourse.mybir as mybir
import concourse.tile as tile
from concourse import bacc
from concourse.bass_utils import run_bass_kernel_spmd
from concourse.masks import make_identity

# run_bass_kernel_spmd imports antenv.axon_hooks when BASS_TRACE is set; that
# module is absent on this image. Provide it so tracing degrades gracefully.
if "antenv.axon_hooks" not in sys.modules:
    try:
        import antenv.axon_hooks  # noqa: F401
    except ImportError:
        _hm = types.ModuleType("antenv.axon_hooks")
        _hm._hook = None
        _hm.set_axon_ntff_profile_hook = lambda h: setattr(_hm, "_hook", h)
        _hm.get_axon_ntff_profile_hook = lambda: _hm._hook
        sys.modules["antenv.axon_hooks"] = _hm
        try:
            from trn_agent_boot.trn_boot import _ntff_profile_via_ctypes

            _hm._hook = _ntff_profile_via_ctypes("/opt/axon/libaxon_pjrt.so")
        except Exception:
            pass

F32 = mybir.dt.float32
F32R = mybir.dt.float32r
BF16 = mybir.dt.bfloat16
I32 = mybir.dt.int32

B, S, D, R, E = 8, 2048, 2048, 512, 8
N_CORES = 8
P = 128
NT = S // P          # 16 token tiles per core
NK = D // P          # 16 contraction chunks
GRP = 4              # stat-processing group (tiles)
LN_EPS = 1e-5

_cache = {}


def _build():
    nc = bacc.Bacc("TRN2", target_bir_lowering=False, debug=False, num_devices=N_CORES)
    xb = nc.dram_tensor("xb", [S, D], BF16, kind="ExternalInput")
    w1g = nc.dram_tensor("w1g", [P, NK * R], BF16, kind="ExternalInput")
    uvb = nc.dram_tensor("uvb", [2, R], BF16, kind="ExternalInput")
    gout = nc.dram_tensor("gout", [P, R], F32, kind="ExternalOutput")

    with tile.TileContext(nc) as tc:
        with (
            tc.tile_pool(name="const", bufs=1) as const,
            tc.tile_pool(name="xin", bufs=3) as xin,
            tc.tile_pool(name="xtp", bufs=2) as xtp,
            tc.tile_pool(name="stats", bufs=2) as stats,
            tc.tile_pool(name="rows", bufs=2) as rows,
            tc.tile_pool(name="work", bufs=3) as work,
            tc.tile_pool(name="pst", bufs=2, space="PSUM") as pst,
            tc.tile_pool(name="psm", bufs=5, space="PSUM") as psm,
            tc.tile_pool(name="pssml", bufs=1, space="PSUM") as pssml,
        ):
            # ---- startup-critical DMA ordering: x tile 0 halves first.
            xs_tiles = [None] * NT
            xs0 = xin.tile([P, NK, P], BF16, tag="xs")
            for q4 in range(4):
                nc.sync.dma_start(
                    xs0[:, q4 * 4 : (q4 + 1) * 4],
                    xb[0:P, q4 * D // 4 : (q4 + 1) * D // 4].rearrange(
                        "p (k q) -> p k q", q=P
                    ),
                )
            xs_tiles[0] = xs0

            ident = const.tile([P, P], BF16)
            make_identity(nc, ident)

            xs1 = xin.tile([P, NK, P], BF16, tag="xs")
            for h2 in range(2):
                nc.sync.dma_start(
                    xs1[:, h2 * 8 : (h2 + 1) * 8],
                    xb[P : 2 * P, h2 * D // 2 : (h2 + 1) * D // 2].rearrange(
                        "p (k q) -> p k q", q=P
                    ),
                )
            xs_tiles[1] = xs1

            # w1g pre-rearranged on host: w1g[p, k*R+n] = w1g_logical[k*128+p, n]
            # (triggered after the first two x tiles so their descriptors win
            # the early engine-bandwidth race; w1gA is only needed ~16us in)
            w1gA = const.tile([P, 4, R], BF16)
            nc.scalar.dma_start(
                w1gA, w1g[:, : 4 * R].rearrange("p (k n) -> p k n", n=R)
            )
            w1gB = const.tile([P, NK - 4, R], BF16)
            nc.scalar.dma_start(
                w1gB, w1g[:, 4 * R :].rearrange("p (k n) -> p k n", n=R)
            )

            def w1g_chunk(k):
                return w1gA[:, k] if k < 4 else w1gB[:, k - 4]

            uvb_s = const.tile([2, R], BF16)
            nc.scalar.dma_start(uvb_s, uvb[:, :])

            gacc = const.tile([P, R], F32)
            nc.vector.memset(gacc, 0.0)

            # ---- per-tile emission, software-pipelined one tile ahead on PE
            mu_g = vr_g = None
            pm_tiles = [None] * NT
            xT_tiles = [None] * NT
            group_ready = [None] * (NT // GRP)   # (rg, row2) per group
            pending = []                         # tiles awaiting K2+postproc

            def emit_front(ti):
                """DMA (ti+2), stats(ti), transposes(ti) + copies(ti)."""
                if ti + 2 < NT:
                    xs_n = xin.tile([P, NK, P], BF16, tag="xs")
                    nc.sync.dma_start(
                        xs_n,
                        xb[(ti + 2) * P : (ti + 3) * P, :].rearrange(
                            "p (k q) -> p k q", q=P
                        ),
                    )
                    xs_tiles[ti + 2] = xs_n
                xs = xs_tiles[ti]

                st = stats.tile([P, 4, 6], F32, tag="bn")
                xs4 = xs.rearrange("p (a k) q -> p a (k q)", a=4)
                for c in range(4):
                    nc.vector.bn_stats(st[:, c], xs4[:, c])
                mv = stats.tile([P, 2], F32, tag="mv")
                nc.vector.bn_aggr(mv, st)
                gi = ti % GRP
                nc.vector.tensor_copy(mu_g[:, gi : gi + 1], mv[:, 0:1])
                nc.vector.tensor_copy(vr_g[:, gi : gi + 1], mv[:, 1:2])

                xT = xtp.tile([P, NK, P], BF16, tag="xT")
                for kg in range(4):
                    pt = pst.tile([P, 4, P], BF16, tag="tp")
                    for j in range(4):
                        k = kg * 4 + j
                        nc.tensor.transpose(pt[:, j], xs[:, k], ident)
                    nc.scalar.copy(xT[:, kg * 4 : (kg + 1) * 4], pt)
                xT_tiles[ti] = xT

            def emit_matmuls(ti):
                pm = psm.tile([P, R], F32, tag="mm")
                xT = xT_tiles[ti]
                for k in range(NK):
                    nc.tensor.matmul(
                        pm, xT[:, k], w1g_chunk(k),
                        start=(k == 0), stop=False, skip_group_check=True,
                    )
                pm_tiles[ti] = pm

            def emit_group_math():
                """rsqrt + stat-row transposes for the current group."""
                v = stats.tile([P, GRP], F32, tag="v")
                nc.vector.tensor_scalar_add(v, vr_g, LN_EPS)
                r = stats.tile([P, GRP], F32, tag="r")
                nc.vector.tensor_scalar(
                    r.bitcast(I32), v.bitcast(I32), 1, None,
                    op0=mybir.AluOpType.arith_shift_right,
                )
                nc.vector.tensor_scalar(
                    r.bitcast(I32), r.bitcast(I32), 0x5F3759DF, -1,
                    op0=mybir.AluOpType.subtract, op1=mybir.AluOpType.mult,
                )
                t = stats.tile([P, GRP], F32, tag="t")
                for _ in range(2):
                    nc.vector.tensor_mul(t, r, r)
                    nc.vector.tensor_mul(t, t, v)
                    nc.vector.tensor_scalar(
                        t, t, -0.5, 1.5,
                        op0=mybir.AluOpType.mult, op1=mybir.AluOpType.add,
                    )
                    nc.vector.tensor_mul(r, r, t)
                rg = stats.tile([P, GRP], F32, tag="rg")
                nc.vector.tensor_copy(rg, r)
                # stationary rows for the K=2 correction: [-mu_j, sqrt(v)_j]
                # (bf16 rows; the correction is small relative to pre, so
                # bf16 rounding of it is ~5e-5 absolute -- negligible).
                nmsq = stats.tile([P, 2, GRP], BF16, tag="nmsq")
                nc.vector.tensor_scalar_mul(nmsq[:, 0], mu_g, -1.0)
                nc.vector.tensor_mul(nmsq[:, 1], v, rg)
                strow = pssml.tile([2, GRP, P], BF16, tag="strow")
                for j in range(GRP):
                    nc.tensor.transpose(strow[:, j], nmsq[:, :, j], ident)
                row2 = rows.tile([2, GRP, P], BF16, tag="row2")
                nc.vector.tensor_copy(row2, strow)
                return rg, row2

            def emit_postproc(ti):
                """K2 rank-2 correction (PE), Gelu*r (ACT), accumulate (DVE)."""
                gi = ti % GRP
                rg, row2 = group_ready[ti // GRP]
                nc.tensor.matmul(
                    pm_tiles[ti], row2[:, gi], uvb_s,
                    start=False, stop=True, skip_group_check=True,
                )
                gt = work.tile([P, R], F32, tag="g")
                nc.scalar.activation(
                    gt, pm_tiles[ti], mybir.ActivationFunctionType.Gelu,
                    scale=rg[:, gi : gi + 1],
                )
                pm_tiles[ti] = None
                nc.vector.tensor_add(gacc, gacc, gt)

            def flush_postproc():
                while pending and group_ready[pending[0] // GRP] is not None:
                    emit_postproc(pending.pop(0))

            for ti in range(NT):
                if ti % GRP == 0:
                    mu_g = stats.tile([P, GRP], F32, tag="mu")
                    vr_g = stats.tile([P, GRP], F32, tag="vr")
                emit_front(ti)
                if ti > 0:
                    emit_matmuls(ti - 1)
                    pending.append(ti - 1)
                if ti % GRP == GRP - 1:
                    group_ready[ti // GRP] = emit_group_math()
                flush_postproc()
            emit_matmuls(NT - 1)
            pending.append(NT - 1)
            flush_postproc()
            assert not pending

            nc.scalar.dma_start(gout[:, :], gacc)
    nc.finalize()
    return nc


def kernel(hidden_states, ln_gamma, ln_beta, w1, b1, w2, b2, wr, br):
    hs = np.asarray(hidden_states, dtype=np.float32)
    xb = np.asarray(hs, dtype=ml_dtypes.bfloat16)            # round-to-nearest
    g64 = np.asarray(ln_gamma, dtype=np.float64)
    be64 = np.asarray(ln_beta, dtype=np.float64)
    w1_64 = np.asarray(w1, dtype=np.float64)
    w1g = (g64[:, None] * w1_64).astype(np.float32)
    w1gb = np.asarray(w1g, dtype=ml_dtypes.bfloat16)
    # u must match the device matmul's weights (bf16) exactly
    u = w1gb.astype(np.float64).sum(0).astype(np.float32)
    # device layout: w1gh[p, k*R+n] = w1g[k*128+p, n]  (16KB contiguous rows)
    w1gh = np.ascontiguousarray(
        w1gb.reshape(16, 128, 512).transpose(1, 0, 2).reshape(128, 16 * 512)
    )
    vb1 = (be64 @ w1_64 + np.asarray(b1, np.float64)).astype(np.float32)
    uvb = np.asarray(np.stack([u, vb1], axis=0), dtype=ml_dtypes.bfloat16)  # [2, R]

    if "nc" not in _cache:
        _cache["nc"] = _build()
    nc = _cache["nc"]

    in_maps = []
    for b in range(N_CORES):
        in_maps.append({
            "xb": np.ascontiguousarray(xb[b]),
            "w1g": w1gh, "uvb": uvb,
        })
    res = run_bass_kernel_spmd(nc, in_maps, core_ids=list(range(N_CORES)))
    gaccs = np.stack([res.results[b]["gout"] for b in range(N_CORES)], axis=0)
    global _last_res
    _last_res = res

    # host tail in fp64 (tiny): H -> w2 -> router -> aux/next_idx
    H = gaccs.astype(np.float64).sum(axis=1)                      # [B, R]
    bt = H @ np.asarray(w2, np.float64) + float(S) * np.asarray(b2, np.float64)
    logits = bt @ np.asarray(wr, np.float64) + np.asarray(br, np.float64)  # [B, E]
    global _last_logits
    _last_logits = logits.astype(np.float32)

    idx = logits.argmax(axis=-1)
    targets = np.zeros_like(logits)
    targets[np.arange(B), idx] = 1.0
    aux = (np.logaddexp(0.0, logits) - logits * targets).mean()
    counts = targets.sum(0)
    next_idx = int(np.argmax(counts))
    return np.float32(aux), np.int32(next_idx)


# revision 26
# speedup vs baseline: 1.2479x; 1.2180x over previous
"""MixtureRouter Trainium2 kernel (final).

Per-core (data-parallel over batch, 8 cores): LayerNorm + Linear(2048->512)
+ GELU + sum-over-sequence, emitting the per-core GELU accumulator
[128, 512] (token-slot x feature). Host sums the 128 token-slot rows,
applies the tiny tail (H @ w2 + S*b2 -> router head) and computes
aux_loss / next_idx in fp64 (trivial 512x512 math, exact rewrite).

Key design points:
  - x is rounded to bf16 on the host (round-to-nearest, symmetric error
    ~2^-9; measured end-to-end logits error ~3e-4 vs the 2e-2 gate).
    bf16 halves HBM traffic (8 MiB/core), makes the PE transposes
    single-pass, and runs the 256 matmuls at the same 1 cyc/row as f32r
    with less power draw (dense fp32 PE work on all 8 cores trips a chip
    power throttle 2.4 -> ~2.0 GHz; measured).
  - LayerNorm folds into the matmul output: with pm = x @ w1g,
      pre = r*(pm - mu (x) u + sqrt(var+eps) (x) vb1)
    i.e. ONE K=2 rank-2 matmul into the same PSUM bank (stationary rows
    [-mu_t, sqrt(v)_t], moving rows [u, vb1]), and the r-scale rides the
    ACT Gelu as a per-partition `scale` operand. No elementwise
    correction passes at all.
  - Stats (mu, var) via DVE bn_stats directly on the bf16 tiles.
  - sum-over-tokens commutes with the second linear:
      sum_t(gelu_h @ w2 + b2) = (sum_t gelu_h) @ w2 + S*b2  (host).
  - rsqrt via DVE bit-trick + 2 Newton steps (avoids ACT table thrash).
  - PE software-pipelined one tile ahead (transposes of tile i+1 issued
    before matmuls of tile i) so the PSUM->SBUF copies (ACT) never stall
    it. w1g is uploaded pre-rearranged so its DMA uses 16KB descriptors.
"""

import sys
import types

import ml_dtypes
import numpy as np

import concourse.bass as bass
import concourse.mybir as mybir
import concourse.tile as tile
from concourse import bacc
from concourse.bass_utils import run_bass_kernel_spmd
from concourse.masks import make_identity

# run_bass_kernel_spmd imports antenv.axon_hooks when BASS_TRACE is set; that
# module is absent on this image. Provide it so tracing degrades gracefully.
if "antenv.axon_hooks" not in sys.modules:
    try:
        import antenv.axon_hooks  # noqa: F401
    except ImportError:
        _hm = types.ModuleType("antenv.axon_hooks")
        _hm._hook = None
        _hm.set_axon_ntff_profile_hook = lambda h: setattr(_hm, "_hook", h)
        _hm.get_axon_ntff_profile_hook = lambda: _hm._hook
        sys.modules["antenv.axon_hooks"] = _hm
        try:
            from trn_agent_boot.trn_boot import _ntff_profile_via_ctypes

            _hm._hook = _ntff_profile_via_ctypes("/opt/axon/libaxon_pjrt.so")
        except Exception:
            pass

F32 = mybir.dt.float32
F32R = mybir.dt.float32r
BF16 = mybir.dt.bfloat16
FP8 = mybir.dt.float8e4
I32 = mybir.dt.int32

B, S, D, R, E = 8, 2048, 2048, 512, 8
N_CORES = 8
P = 128
NT = S // P          # 16 token tiles per core
NK = D // P          # 16 contraction chunks
GRP = 4              # stat-processing group (tiles)
LN_EPS = 1e-5

_cache = {}


def _build():
    nc = bacc.Bacc("TRN2", target_bir_lowering=False, debug=False, num_devices=N_CORES)
    xb = nc.dram_tensor("xb", [S, D], BF16, kind="ExternalInput")
    w1g = nc.dram_tensor("w1g", [P, NK * R], FP8, kind="ExternalInput")
    uvb = nc.dram_tensor("uvb", [2, R], BF16, kind="ExternalInput")
    gout = nc.dram_tensor("gout", [P, R], F32, kind="ExternalOutput")

    with tile.TileContext(nc) as tc:
        with (
            tc.tile_pool(name="const", bufs=1) as const,
            tc.tile_pool(name="xin", bufs=3) as xin,
            tc.tile_pool(name="xtp", bufs=2) as xtp,
            tc.tile_pool(name="stats", bufs=2) as stats,
            tc.tile_pool(name="rows", bufs=2) as rows,
            tc.tile_pool(name="work", bufs=3) as work,
            tc.tile_pool(name="pst", bufs=2, space="PSUM") as pst,
            tc.tile_pool(name="psm", bufs=5, space="PSUM") as psm,
            tc.tile_pool(name="pssml", bufs=1, space="PSUM") as pssml,
        ):
            # ---- startup-critical DMA ordering: x tile 0 halves first.
            xs_tiles = [None] * NT
            xs0 = xin.tile([P, NK, P], BF16, tag="xs")
            for q4 in range(4):
                nc.sync.dma_start(
                    xs0[:, q4 * 4 : (q4 + 1) * 4],
                    xb[0:P, q4 * D // 4 : (q4 + 1) * D // 4].rearrange(
                        "p (k q) -> p k q", q=P
                    ),
                )
            xs_tiles[0] = xs0

            ident = const.tile([P, P], BF16)
            make_identity(nc, ident)

            xs1 = xin.tile([P, NK, P], BF16, tag="xs")
            for h2 in range(2):
                nc.sync.dma_start(
                    xs1[:, h2 * 8 : (h2 + 1) * 8],
                    xb[P : 2 * P, h2 * D // 2 : (h2 + 1) * D // 2].rearrange(
                        "p (k q) -> p k q", q=P
                    ),
                )
            xs_tiles[1] = xs1

            # w1g pre-rearranged on host: w1g[p, k*R+n] = w1g_logical[k*128+p, n]
            # (triggered after the first two x tiles so their descriptors win
            # the early engine-bandwidth race; w1gA is only needed ~16us in)
            w1gA = const.tile([P, 4, R], FP8)
            nc.scalar.dma_start(
                w1gA, w1g[:, : 4 * R].rearrange("p (k n) -> p k n", n=R)
            )
            w1gB = const.tile([P, NK - 4, R], FP8)
            nc.scalar.dma_start(
                w1gB, w1g[:, 4 * R :].rearrange("p (k n) -> p k n", n=R)
            )

            def w1g_pair(k2):
                # [128, 2, 512] slice spanning k-subtiles (k2, k2+1)
                return (
                    w1gA[:, k2 : k2 + 2]
                    if k2 < 4
                    else w1gB[:, k2 - 4 : k2 - 2]
                )

            uvb_s = const.tile([2, R], BF16)
            nc.scalar.dma_start(uvb_s, uvb[:, :])

            gacc = const.tile([P, R], F32)
            nc.vector.memset(gacc, 0.0)

            # ---- per-tile emission, software-pipelined one tile ahead on PE
            mu_g = vr_g = None
            pm_tiles = [None] * NT
            xT_tiles = [None] * NT
            group_ready = [None] * (NT // GRP)   # (rg, row2) per group
            pending = []                         # tiles awaiting K2+postproc

            def emit_front(ti):
                """DMA (ti+2), stats(ti), transposes(ti) + copies(ti)."""
                if ti + 2 < NT:
                    xs_n = xin.tile([P, NK, P], BF16, tag="xs")
                    nc.sync.dma_start(
                        xs_n,
                        xb[(ti + 2) * P : (ti + 3) * P, :].rearrange(
                            "p (k q) -> p k q", q=P
                        ),
                    )
                    xs_tiles[ti + 2] = xs_n
                xs = xs_tiles[ti]

                st = stats.tile([P, 4, 6], F32, tag="bn")
                xs4 = xs.rearrange("p (a k) q -> p a (k q)", a=4)
                for c in range(4):
                    nc.vector.bn_stats(st[:, c], xs4[:, c])
                mv = stats.tile([P, 2], F32, tag="mv")
                nc.vector.bn_aggr(mv, st)
                gi = ti % GRP
                nc.vector.tensor_copy(mu_g[:, gi : gi + 1], mv[:, 0:1])
                nc.vector.tensor_copy(vr_g[:, gi : gi + 1], mv[:, 1:2])

                xT = xtp.tile([P, NK, P], FP8, tag="xT")
                for kg in range(4):
                    pt = pst.tile([P, 4, P], BF16, tag="tp")
                    for j in range(4):
                        k = kg * 4 + j
                        nc.tensor.transpose(pt[:, j], xs[:, k], ident)
                    nc.scalar.copy(xT[:, kg * 4 : (kg + 1) * 4], pt)
                xT_tiles[ti] = xT

            def emit_matmuls(ti):
                pm = psm.tile([P, R], F32, tag="mm")
                xT = xT_tiles[ti]
                for k2 in range(0, NK, 2):
                    nc.tensor.matmul(
                        pm, xT[:, k2 : k2 + 2], w1g_pair(k2),
                        start=(k2 == 0), stop=False, skip_group_check=True,
                        perf_mode=mybir.MatmulPerfMode.DoubleRow,
                    )
                pm_tiles[ti] = pm

            def emit_group_math():
                """rsqrt + stat-row transposes for the current group."""
                v = stats.tile([P, GRP], F32, tag="v")
                nc.vector.tensor_scalar_add(v, vr_g, LN_EPS)
                r = stats.tile([P, GRP], F32, tag="r")
                nc.vector.tensor_scalar(
                    r.bitcast(I32), v.bitcast(I32), 1, None,
                    op0=mybir.AluOpType.arith_shift_right,
                )
                nc.vector.tensor_scalar(
                    r.bitcast(I32), r.bitcast(I32), 0x5F3759DF, -1,
                    op0=mybir.AluOpType.subtract, op1=mybir.AluOpType.mult,
                )
                t = stats.tile([P, GRP], F32, tag="t")
                for _ in range(2):
                    nc.vector.tensor_mul(t, r, r)
                    nc.vector.tensor_mul(t, t, v)
                    nc.vector.tensor_scalar(
                        t, t, -0.5, 1.5,
                        op0=mybir.AluOpType.mult, op1=mybir.AluOpType.add,
                    )
                    nc.vector.tensor_mul(r, r, t)
                rg = stats.tile([P, GRP], F32, tag="rg")
                nc.vector.tensor_copy(rg, r)
                # stationary rows for the K=2 correction: [-mu_j, sqrt(v)_j]
                # (bf16 rows; the correction is small relative to pre, so
                # bf16 rounding of it is ~5e-5 absolute -- negligible).
                nmsq = stats.tile([P, 2, GRP], BF16, tag="nmsq")
                nc.vector.tensor_scalar_mul(nmsq[:, 0], mu_g, -1.0)
                nc.vector.tensor_mul(nmsq[:, 1], v, rg)
                strow = pssml.tile([2, GRP, P], BF16, tag="strow")
                for j in range(GRP):
                    nc.tensor.transpose(strow[:, j], nmsq[:, :, j], ident)
                row2 = rows.tile([2, GRP, P], BF16, tag="row2")
                nc.vector.tensor_copy(row2, strow)
                return rg, row2

            def emit_postproc(ti):
                """K2 rank-2 correction (PE), Gelu*r (ACT), accumulate (DVE)."""
                gi = ti % GRP
                rg, row2 = group_ready[ti // GRP]
                nc.tensor.matmul(
                    pm_tiles[ti], row2[:, gi], uvb_s,
                    start=False, stop=True, skip_group_check=True,
                )
                gt = work.tile([P, R], F32, tag="g")
                nc.scalar.activation(
                    gt, pm_tiles[ti], mybir.ActivationFunctionType.Gelu,
                    scale=rg[:, gi : gi + 1],
                )
                pm_tiles[ti] = None
                nc.vector.tensor_add(gacc, gacc, gt)

            def flush_postproc():
                while pending and group_ready[pending[0] // GRP] is not None:
                    emit_postproc(pending.pop(0))

            for ti in range(NT):
                if ti % GRP == 0:
                    mu_g = stats.tile([P, GRP], F32, tag="mu")
                    vr_g = stats.tile([P, GRP], F32, tag="vr")
                emit_front(ti)
                if ti > 0:
                    emit_matmuls(ti - 1)
                    pending.append(ti - 1)
                if ti % GRP == GRP - 1:
                    group_ready[ti // GRP] = emit_group_math()
                flush_postproc()
            emit_matmuls(NT - 1)
            pending.append(NT - 1)
            flush_postproc()
            assert not pending

            nc.scalar.dma_start(gout[:, :], gacc)
    nc.finalize()
    return nc


def kernel(hidden_states, ln_gamma, ln_beta, w1, b1, w2, b2, wr, br):
    hs = np.asarray(hidden_states, dtype=np.float32)
    xb = np.asarray(hs, dtype=ml_dtypes.bfloat16)            # round-to-nearest
    g64 = np.asarray(ln_gamma, dtype=np.float64)
    be64 = np.asarray(ln_beta, dtype=np.float64)
    w1_64 = np.asarray(w1, dtype=np.float64)
    w1g = (g64[:, None] * w1_64).astype(np.float32)
    w1gb = np.asarray(w1g, dtype=ml_dtypes.float8_e4m3fn)
    # u must match the device matmul's weights (fp8) exactly
    u = w1gb.astype(np.float64).sum(0).astype(np.float32)
    # device layout: w1gh[p, k*R+n] = w1g[k*128+p, n]  (8KB contiguous rows)
    w1gh = np.ascontiguousarray(
        w1gb.reshape(16, 128, 512).transpose(1, 0, 2).reshape(128, 16 * 512)
    )
    vb1 = (be64 @ w1_64 + np.asarray(b1, np.float64)).astype(np.float32)
    uvb = np.asarray(np.stack([u, vb1], axis=0), dtype=ml_dtypes.bfloat16)  # [2, R]

    if "nc" not in _cache:
        _cache["nc"] = _build()
    nc = _cache["nc"]

    in_maps = []
    for b in range(N_CORES):
        in_maps.append({
            "xb": np.ascontiguousarray(xb[b]),
            "w1g": w1gh, "uvb": uvb,
        })
    res = run_bass_kernel_spmd(nc, in_maps, core_ids=list(range(N_CORES)))
    gaccs = np.stack([res.results[b]["gout"] for b in range(N_CORES)], axis=0)
    global _last_res
    _last_res = res

    # host tail in fp64 (tiny): H -> w2 -> router -> aux/next_idx
    H = gaccs.astype(np.float64).sum(axis=1)                      # [B, R]
    bt = H @ np.asarray(w2, np.float64) + float(S) * np.asarray(b2, np.float64)
    logits = bt @ np.asarray(wr, np.float64) + np.asarray(br, np.float64)  # [B, E]
    global _last_logits
    _last_logits = logits.astype(np.float32)

    idx = logits.argmax(axis=-1)
    targets = np.zeros_like(logits)
    targets[np.arange(B), idx] = 1.0
    aux = (np.logaddexp(0.0, logits) - logits * targets).mean()
    counts = targets.sum(0)
    next_idx = int(np.argmax(counts))
    return np.float32(aux), np.int32(next_idx)
